# revision 9
# baseline (speedup 1.0000x reference)
"""Causal multi-head attention (B=4, S=2048, D=1024, H=16) on 8 NeuronCores.

Sharding: core c = (batch b = c//2, head-group hg = c%2). Each core computes
8 heads of one batch: QKV projection (bf16 matmuls), causal flash-style
attention (bf16 matmuls, exp-without-max softmax with a ones-column
denominator), and a row-parallel out-projection partial. Host sums the two
bf16 head-group partials per batch, adds bias, and transposes.

All HBM payloads are bf16. Layouts are feature-major ([feature, token])
except v (token-major) so attn@v needs no transposes. Head pairs are packed
into PE row groups (rows 0-63 / 64-127): the two K=64 score matmuls run
concurrently on PE row-tiles T0/T8. PSUM score tiles are 2 banks wide
(even head in columns 0-511, odd in 512-1023) so one ACT exp covers both
heads. Out-projection runs K=128 matmuls accumulating all 4 head pairs in
PSUM.

Scheduling: ACT exp throughput (~1µs per score tile) is the attention-phase
bottleneck, so projection/out-proj work is chopped into small filler units
(8 or 4 matmuls) and pumped between attention kt-tiles to fill the PE's
exp-wait gaps. DMA descriptor generation is spread across sync (weights)
and gpsimd (x, outputs) queues.
"""
import numpy as np
from collections import deque
from contextlib import ExitStack

import ml_dtypes

B, S, D, H = 4, 2048, 1024, 16
HD = 64            # head dim
HPC = 8            # heads per core
F = HPC * HD       # 512 features per head-group
QT = 512           # q tile (free dim)
NQI = S // QT      # 4
NKT = S // 128     # 16
NDK = D // 128     # 8 contraction tiles for projections
SCALE = HD ** -0.5

_CACHE = {}


def _build():
    import concourse.bacc as bacc
    import concourse.tile as tile
    import concourse.mybir as mybir

    f32 = mybir.dt.float32
    bf16 = mybir.dt.bfloat16
    EXP = mybir.ActivationFunctionType.Exp

    nc = bacc.Bacc("TRN2", target_bir_lowering=False, debug=False)
    xT = nc.dram_tensor("xT", [D, S], bf16, kind="ExternalInput").ap()
    w_sl = nc.dram_tensor("w_sl", [D, 3 * F], bf16, kind="ExternalInput").ap()
    wo_sl = nc.dram_tensor("wo_sl", [F, D], bf16, kind="ExternalInput").ap()
    mask2 = nc.dram_tensor("mask2", [128, 256], bf16, kind="ExternalInput").ap()
    out = nc.dram_tensor("out", [D, S], bf16, kind="ExternalOutput").ap()

    with tile.TileContext(nc) as tc:
        with ExitStack() as ctx:
            misc = ctx.enter_context(tc.tile_pool(name="misc", bufs=1))
            mask_sb = misc.tile([128, 256], bf16, name="mask_sb", tag="mask")
            nc.sync.dma_start(mask_sb[:], mask2)

            pqk = ctx.enter_context(tc.tile_pool(name="pqk", bufs=1))
            pv = ctx.enter_context(tc.tile_pool(name="pv", bufs=1))
            patt = ctx.enter_context(tc.tile_pool(name="patt", bufs=16))
            pP = ctx.enter_context(tc.tile_pool(name="pP", bufs=3))
            pr = ctx.enter_context(tc.tile_pool(name="pr", bufs=2))
            prr = ctx.enter_context(tc.tile_pool(name="prr", bufs=1))
            pwo = ctx.enter_context(tc.tile_pool(name="pwo", bufs=1))
            pstg = ctx.enter_context(tc.tile_pool(name="pstg", bufs=3))
            xw = ctx.enter_context(tc.tile_pool(name="xw", bufs=1))

            q_sb = [pqk.tile([128, S], bf16, name=f"q{g}", tag=f"q{g}")
                    for g in range(4)]
            k_sb = [pqk.tile([128, S], bf16, name=f"k{g}", tag=f"k{g}")
                    for g in range(4)]
            v_sb = [pv.tile([128, HPC * (HD + 1)], bf16, name=f"v{t}",
                            tag=f"v{t}") for t in range(NKT)]

            psum = ctx.enter_context(
                tc.tile_pool(name="psum", bufs=2, space="PSUM"))

            # ---- input DMA, ordered by first use; x on the gpsimd queue,
            # weights on sync, so descriptor gen runs in parallel ----
            wv_t = []
            x_t = []
            wqk_t = []
            for kk in range(NDK):
                r0 = slice(kk * 128, (kk + 1) * 128)
                wv = xw.tile([128, F], bf16, name=f"wv{kk}", tag=f"wv{kk}")
                nc.sync.dma_start(wv[:], w_sl[r0, 2 * F:3 * F])
                wv_t.append(wv)
                x_t.append(xw.tile([128, S], bf16, name=f"x{kk}",
                                   tag=f"x{kk}"))
            for ch in range(4):
                cs = slice(ch * QT, (ch + 1) * QT)
                for kk in range(NDK):
                    r0 = slice(kk * 128, (kk + 1) * 128)
                    nc.gpsimd.dma_start(x_t[kk][:, cs], xT[r0, cs])
                if ch == 1:
                    for kk in range(NDK):
                        r0 = slice(kk * 128, (kk + 1) * 128)
                        wq = xw.tile([128, 2 * F], bf16, name=f"wq{kk}",
                                     tag=f"wq{kk}")
                        nc.sync.dma_start(wq[:], w_sl[r0, 0:2 * F])
                        wqk_t.append(wq)
            wo_t = []
            for g in range(4):
                wt = pwo.tile([128, D], bf16, name=f"wo{g}", tag=f"wo{g}")
                nc.sync.dma_start(wt[:], wo_sl[g * 128:(g + 1) * 128, :])
                wo_t.append(wt)

            att_m = {}

            # ---- filler units: small PE chunks pumped into exp-wait gaps.
            # Each unit is (cost_ns, emit_fn); pump() spends a time budget.
            fillers = deque()
            state = {"budget": 0.0}

            def pump(ns):
                state["budget"] += ns
                while fillers and state["budget"] >= fillers[0][0]:
                    cost, emit = fillers.popleft()
                    state["budget"] -= cost
                    emit()

            def v_unit(t2, j):
                def emit():
                    tt = 2 * t2 + j
                    ps = psum.tile([128, QT], f32, name=f"pv{tt}", tag="sc")
                    for kk in range(NDK):
                        nc.tensor.matmul(
                            ps[:], x_t[kk][:, tt * 128:(tt + 1) * 128],
                            wv_t[kk][:],
                            start=(kk == 0), stop=(kk == NDK - 1))
                    vv = v_sb[tt].rearrange("p (h c) -> p h c", h=HPC)
                    pp = ps.rearrange("p (h c) -> p h c", h=HPC)
                    nc.vector.tensor_copy(vv[:, :, 0:HD], pp[:])
                    nc.vector.memset(vv[:, :, HD:HD + 1], 1.0)
                return emit

            def qk_unit(g, part, tg):
                def emit():
                    dest = q_sb if part == 0 else k_sb
                    fcol = part * F + g * 128
                    ps = psum.tile([128, QT], f32,
                                   name=f"pq{part}{g}{tg}", tag="sc")
                    for kk in range(NDK):
                        nc.tensor.matmul(
                            ps[:], wqk_t[kk][:, fcol:fcol + 128],
                            x_t[kk][:, tg * QT:(tg + 1) * QT],
                            start=(kk == 0), stop=(kk == NDK - 1))
                    nc.vector.tensor_copy(
                        dest[g][:, tg * QT:(tg + 1) * QT], ps[:])
                return emit

            def op_unit(qi, dt):
                def emit():
                    dcol = slice(dt * 128, dt * 128 + 128)
                    ps = psum.tile([128, QT], f32,
                                   name=f"op{dt}{qi}", tag="sc")
                    for pg in range(4):
                        nc.tensor.matmul(
                            ps[:], wo_t[pg][:, dcol], att_m[(pg, qi)][:],
                            start=(pg == 0), stop=(pg == 3))
                    s2 = pstg.tile([128, QT], bf16, name=f"s2{dt}{qi}",
                                   tag="s2")
                    nc.vector.tensor_copy(s2[:], ps[:])
                    nc.gpsimd.dma_start(
                        out[dt * 128:(dt + 1) * 128,
                            qi * QT:(qi + 1) * QT], s2[:])
                return emit

            def attn_block(pg, qi, rate=700):
                """Scores + exp + attn@v + normalize for head pair pg,
                q-range [qi*QT, (qi+1)*QT). Pumps ~rate ns of filler per
                kt tile into the PE's exp-wait gap (between the score
                matmuls and the exp-dependent attn@v matmuls)."""
                nkt = 4 * qi + 4
                qs = qi * QT
                he, ho = 2 * pg, 2 * pg + 1
                C = HD + 1
                ao = psum.tile([HD + 1, 2 * QT], f32,
                               name=f"ao{pg}{qi}", tag="ao")
                for kt in range(nkt):
                    d = kt - 4 * qi
                    n0 = 0 if d < 0 else 128 * d
                    kcol = slice(kt * 128, kt * 128 + 128)
                    sc = psum.tile([128, 2 * QT], f32,
                                   name=f"sc{pg}{qi}{kt}", tag="sc")
                    nc.tensor.matmul(
                        sc[:, n0:QT], k_sb[pg][0:64, kcol],
                        q_sb[pg][0:64, qs + n0:qs + QT],
                        start=True, stop=True)
                    nc.tensor.matmul(
                        sc[:, QT + n0:2 * QT], k_sb[pg][64:128, kcol],
                        q_sb[pg][64:128, qs + n0:qs + QT],
                        start=True, stop=True)
                    pt = pP.tile([128, 2 * QT], bf16,
                                 name=f"pt{pg}{qi}{kt}", tag="P")
                    sc3 = sc.rearrange("p (h c) -> p h c", h=2)
                    pt3 = pt.rearrange("p (h c) -> p h c", h=2)
                    nc.scalar.activation(pt3[:, :, n0:QT], sc3[:, :, n0:QT],
                                         EXP, scale=SCALE)
                    if d >= 0:
                        m3 = mask_sb.rearrange("p (h c) -> p h c", h=2)
                        nc.vector.tensor_mul(pt3[:, :, n0:n0 + 128],
                                             pt3[:, :, n0:n0 + 128], m3[:])
                    pump(rate)
                    st = (kt == 0)
                    sp = (kt == nkt - 1)
                    nc.tensor.matmul(
                        ao[:, n0:QT], v_sb[kt][:, he * C:(he + 1) * C],
                        pt[:, n0:QT], start=st, stop=sp)
                    nc.tensor.matmul(
                        ao[:, QT + n0:2 * QT],
                        v_sb[kt][:, ho * C:(ho + 1) * C],
                        pt[:, QT + n0:2 * QT], start=st, stop=sp)

                # normalize: 1/rowsum (row HD) via fast recip + gpsimd bcast
                am = patt.tile([128, QT], bf16, name=f"am{pg}{qi}", tag="am")
                att_m[(pg, qi)] = am
                srow = prr.tile([1, 2 * QT], f32, name=f"sr{pg}{qi}", tag="sr")
                nc.vector.tensor_copy(srow[:], ao[HD:HD + 1, :])
                nc.vector.reciprocal_approx_fast(srow[:], srow[:])
                rb = pr.tile([HD, 2 * QT], f32, name=f"rb{pg}{qi}", tag="r")
                nc.gpsimd.partition_broadcast(rb[:], srow[:], channels=HD)
                nc.vector.tensor_mul(am[0:64, :], ao[0:HD, 0:QT], rb[:, 0:QT])
                nc.vector.tensor_mul(am[64:128, :], ao[0:HD, QT:2 * QT],
                                     rb[:, QT:2 * QT])

            # ---- emission schedule ----
            # Prologue: v for kt 0-5 and q/k token-quarter 0 of pair 0, so
            # attn(0,0) can start; everything else becomes filler.
            for u in (v_unit(0, 0), v_unit(0, 1), v_unit(1, 0), v_unit(1, 1),
                      v_unit(2, 0), v_unit(2, 1),
                      qk_unit(0, 0, 0), qk_unit(0, 1, 0)):
                u()

            QKV_NS = 1750
            OP_NS = 900
            # Fillers in dependency order: v tiles ahead of the attn(0,*)
            # blocks that need them, then q/k quarters for later pairs.
            fillers.extend((QKV_NS, qk_unit(0, p, 1)) for p in (0, 1))
            fillers.extend((QKV_NS, v_unit(3, j)) for j in (0, 1))
            fillers.extend((QKV_NS, qk_unit(0, p, 2)) for p in (0, 1))
            fillers.extend((QKV_NS, v_unit(t2, j))
                           for t2 in (4, 5) for j in (0, 1))
            fillers.extend((QKV_NS, qk_unit(0, p, 3)) for p in (0, 1))
            fillers.extend((QKV_NS, v_unit(t2, j))
                           for t2 in (6, 7) for j in (0, 1))
            for g in (1, 2, 3):
                fillers.extend((QKV_NS, qk_unit(g, p, tg))
                               for tg in range(4) for p in (0, 1))

            for qi in range(NQI):
                attn_block(0, qi, rate=900)
            for g in (1, 2):
                for qi in range(NQI):
                    attn_block(g, qi, rate=700)
            for qi in range(NQI):
                attn_block(3, qi, rate=750)
                fillers.extend((OP_NS, op_unit(qi, dt)) for dt in range(8))
            while fillers:
                fillers.popleft()[1]()

    nc.compile()
    return nc


def _get_nc():
    if "nc" not in _CACHE:
        _CACHE["nc"] = _build()
    return _CACHE["nc"]


def _prep_inputs(x, w_qkv, w_out, b_out):
    """Build the 8 per-core input maps (all payloads bf16)."""
    bf = ml_dtypes.bfloat16
    x = np.asarray(x, dtype=np.float32)
    w_qkv = np.asarray(w_qkv, dtype=np.float32)
    w_out = np.asarray(w_out, dtype=np.float32)

    tri = np.triu(np.ones((128, 128), dtype=np.float32))
    mask2 = np.tile(tri, (1, 2)).astype(bf)

    in_maps = []
    for c in range(8):
        b, hg = c // 2, c % 2
        cols = hg * F
        w_cat = np.concatenate([
            w_qkv[:, cols:cols + F],
            w_qkv[:, D + cols:D + cols + F],
            w_qkv[:, 2 * D + cols:2 * D + cols + F],
        ], axis=1)
        in_maps.append({
            "xT": np.ascontiguousarray(x[b].T).astype(bf),
            "w_sl": np.ascontiguousarray(w_cat).astype(bf),
            "wo_sl": np.ascontiguousarray(w_out[cols:cols + F, :]).astype(bf),
            "mask2": mask2,
        })
    return in_maps


def _run(inputs, trace=False):
    from concourse.bass_utils import run_bass_kernel_spmd

    nc = _get_nc()
    in_maps = _prep_inputs(**inputs)
    res = run_bass_kernel_spmd(nc, in_maps, core_ids=list(range(8)),
                               trace=trace)
    b_out = np.asarray(inputs["b_out"], dtype=np.float32)
    outs = []
    for b in range(B):
        o = (res.results[2 * b]["out"].astype(np.float32)
             + res.results[2 * b + 1]["out"].astype(np.float32))
        outs.append(o.T + b_out)
    full = np.stack(outs).astype(np.float32)
    return full, res


def kernel(x, w_qkv, w_out, b_out):
    full, _ = _run({"x": x, "w_qkv": w_qkv, "w_out": w_out, "b_out": b_out})
    return full


# revision 16
# speedup vs baseline: 1.0691x; 1.0691x over previous
"""Causal multi-head attention (B=4, S=2048, D=1024, H=16) on 8 NeuronCores.

Sharding: core c = (batch b = c//2, head-group hg = c%2). Each core computes
8 heads of one batch: QKV projection (bf16 matmuls), causal flash-style
attention (bf16 matmuls, exp-without-max softmax with a ones-column
denominator), and a row-parallel out-projection partial. Host sums the two
bf16 head-group partials per batch, adds bias, and transposes.

All HBM payloads are bf16. Layouts are feature-major ([feature, token])
except v (token-major) so attn@v needs no transposes. Head pairs are packed
into PE row groups (rows 0-63 / 64-127): the two K=64 score matmuls run
concurrently on PE row-tiles T0/T8. PSUM score tiles are 2 banks wide
(even head in columns 0-511, odd in 512-1023) so one ACT exp covers both
heads. Out-projection runs K=128 matmuls accumulating all 4 head pairs in
PSUM.

Scheduling: ACT exp throughput (~1µs per score tile) is the attention-phase
bottleneck, so projection/out-proj work is chopped into small filler units
(8 or 4 matmuls) and pumped between attention kt-tiles to fill the PE's
exp-wait gaps. DMA descriptor generation is spread across sync (weights)
and gpsimd (x, outputs) queues.
"""
import numpy as np

from contextlib import ExitStack

import ml_dtypes

B, S, D, H = 4, 2048, 1024, 16
HD = 64            # head dim
HPC = 8            # heads per core
F = HPC * HD       # 512 features per head-group
QT = 512           # q tile (free dim)
NQI = S // QT      # 4
NKT = S // 128     # 16
NDK = D // 128     # 8 contraction tiles for projections
SCALE = HD ** -0.5

_CACHE = {}


def _build():
    import concourse.bacc as bacc
    import concourse.tile as tile
    import concourse.mybir as mybir

    f32 = mybir.dt.float32
    bf16 = mybir.dt.bfloat16
    EXP = mybir.ActivationFunctionType.Exp

    nc = bacc.Bacc("TRN2", target_bir_lowering=False, debug=False)
    xT = nc.dram_tensor("xT", [D, S], bf16, kind="ExternalInput").ap()
    w_sl = nc.dram_tensor("w_sl", [D, 3 * F], bf16, kind="ExternalInput").ap()
    wo_sl = nc.dram_tensor("wo_sl", [F, D], bf16, kind="ExternalInput").ap()
    mask2 = nc.dram_tensor("mask2", [128, 256], bf16, kind="ExternalInput").ap()
    out = nc.dram_tensor("out", [D, S], bf16, kind="ExternalOutput").ap()

    with tile.TileContext(nc) as tc:
        with ExitStack() as ctx:
            misc = ctx.enter_context(tc.tile_pool(name="misc", bufs=1))
            mask_sb = misc.tile([128, 256], bf16, name="mask_sb", tag="mask")
            nc.sync.dma_start(mask_sb[:], mask2)

            pqk = ctx.enter_context(tc.tile_pool(name="pqk", bufs=1))
            pv = ctx.enter_context(tc.tile_pool(name="pv", bufs=1))
            patt = ctx.enter_context(tc.tile_pool(name="patt", bufs=16))
            pP = ctx.enter_context(tc.tile_pool(name="pP", bufs=4))
            pr = ctx.enter_context(tc.tile_pool(name="pr", bufs=2))
            prr = ctx.enter_context(tc.tile_pool(name="prr", bufs=1))
            pwo = ctx.enter_context(tc.tile_pool(name="pwo", bufs=1))
            pstg = ctx.enter_context(tc.tile_pool(name="pstg", bufs=3))
            xw = ctx.enter_context(tc.tile_pool(name="xw", bufs=1))

            q_sb = [pqk.tile([128, S], bf16, name=f"q{g}", tag=f"q{g}")
                    for g in range(4)]
            k_sb = [pqk.tile([128, S], bf16, name=f"k{g}", tag=f"k{g}")
                    for g in range(4)]
            v_sb = [pv.tile([128, HPC * (HD + 1)], bf16, name=f"v{t}",
                            tag=f"v{t}") for t in range(NKT)]

            psum = ctx.enter_context(
                tc.tile_pool(name="psum", bufs=2, space="PSUM"))

            # ---- input DMA, ordered by first use; x on the gpsimd queue,
            # weights on sync, so descriptor gen runs in parallel ----
            wv_t = []
            x_t = []
            wqk_t = []
            for kk in range(NDK):
                r0 = slice(kk * 128, (kk + 1) * 128)
                wv = xw.tile([128, F], bf16, name=f"wv{kk}", tag=f"wv{kk}")
                nc.sync.dma_start(wv[:], w_sl[r0, 2 * F:3 * F])
                wv_t.append(wv)
                x_t.append(xw.tile([128, S], bf16, name=f"x{kk}",
                                   tag=f"x{kk}"))
            for ch in range(4):
                cs = slice(ch * QT, (ch + 1) * QT)
                for kk in range(NDK):
                    r0 = slice(kk * 128, (kk + 1) * 128)
                    nc.gpsimd.dma_start(x_t[kk][:, cs], xT[r0, cs])
                if ch == 1:
                    for kk in range(NDK):
                        r0 = slice(kk * 128, (kk + 1) * 128)
                        wq = xw.tile([128, 2 * F], bf16, name=f"wq{kk}",
                                     tag=f"wq{kk}")
                        nc.sync.dma_start(wq[:], w_sl[r0, 0:2 * F])
                        wqk_t.append(wq)
            wo_t = []
            for g in range(4):
                wt = pwo.tile([128, D], bf16, name=f"wo{g}", tag=f"wo{g}")
                nc.sync.dma_start(wt[:], wo_sl[g * 128:(g + 1) * 128, :])
                wo_t.append(wt)

            att_m = {}

            # ---- projection/out-proj work, chopped into units that slot
            # between attention blocks ----

            def v_unit(t2, j):
                def emit():
                    tt = 2 * t2 + j
                    ps = psum.tile([128, QT], f32, name=f"pv{tt}", tag="sc")
                    for kk in range(NDK):
                        nc.tensor.matmul(
                            ps[:], x_t[kk][:, tt * 128:(tt + 1) * 128],
                            wv_t[kk][:],
                            start=(kk == 0), stop=(kk == NDK - 1))
                    vv = v_sb[tt].rearrange("p (h c) -> p h c", h=HPC)
                    pp = ps.rearrange("p (h c) -> p h c", h=HPC)
                    nc.vector.tensor_copy(vv[:, :, 0:HD], pp[:])
                    nc.vector.memset(vv[:, :, HD:HD + 1], 1.0)
                return emit

            def qk_unit(g, part, tg):
                def emit():
                    dest = q_sb if part == 0 else k_sb
                    fcol = part * F + g * 128
                    ps = psum.tile([128, QT], f32,
                                   name=f"pq{part}{g}{tg}", tag="sc")
                    for kk in range(NDK):
                        nc.tensor.matmul(
                            ps[:], wqk_t[kk][:, fcol:fcol + 128],
                            x_t[kk][:, tg * QT:(tg + 1) * QT],
                            start=(kk == 0), stop=(kk == NDK - 1))
                    nc.vector.tensor_copy(
                        dest[g][:, tg * QT:(tg + 1) * QT], ps[:])
                return emit

            def op_unit(qi, dt):
                def emit():
                    dcol = slice(dt * 128, dt * 128 + 128)
                    ps = psum.tile([128, QT], f32,
                                   name=f"op{dt}{qi}", tag="sc")
                    for pg in range(4):
                        nc.tensor.matmul(
                            ps[:], wo_t[pg][:, dcol], att_m[(pg, qi)][:],
                            start=(pg == 0), stop=(pg == 3))
                    s2 = pstg.tile([128, QT], bf16, name=f"s2{dt}{qi}",
                                   tag="s2")
                    nc.vector.tensor_copy(s2[:], ps[:])
                    nc.gpsimd.dma_start(
                        out[dt * 128:(dt + 1) * 128,
                            qi * QT:(qi + 1) * QT], s2[:])
                return emit

            def attn_block(pg, qi):
                """Scores + exp + attn@v + normalize for head pair pg,
                q-range [qi*QT, (qi+1)*QT). attn@v for tile kt is emitted
                after the scores+exp of tile kt+1, so it never waits on
                the exp result (one-tile softmax lag)."""
                nkt = 4 * qi + 4
                qs = qi * QT
                he, ho = 2 * pg, 2 * pg + 1
                C = HD + 1
                ao = psum.tile([HD + 1, 2 * QT], f32,
                               name=f"ao{pg}{qi}", tag="ao")
                pend = None
                for kt in range(nkt):
                    d = kt - 4 * qi
                    n0 = 0 if d < 0 else 128 * d
                    kcol = slice(kt * 128, kt * 128 + 128)
                    sc = psum.tile([128, 2 * QT], f32,
                                   name=f"sc{pg}{qi}{kt}", tag="sc")
                    nc.tensor.matmul(
                        sc[:, n0:QT], k_sb[pg][0:64, kcol],
                        q_sb[pg][0:64, qs + n0:qs + QT],
                        start=True, stop=True)
                    nc.tensor.matmul(
                        sc[:, QT + n0:2 * QT], k_sb[pg][64:128, kcol],
                        q_sb[pg][64:128, qs + n0:qs + QT],
                        start=True, stop=True)
                    pt = pP.tile([128, 2 * QT], bf16,
                                 name=f"pt{pg}{qi}{kt}", tag="P")
                    sc3 = sc.rearrange("p (h c) -> p h c", h=2)
                    pt3 = pt.rearrange("p (h c) -> p h c", h=2)
                    nc.scalar.activation(pt3[:, :, n0:QT], sc3[:, :, n0:QT],
                                         EXP, scale=SCALE)
                    if d >= 0:
                        m3 = mask_sb.rearrange("p (h c) -> p h c", h=2)
                        nc.vector.tensor_mul(pt3[:, :, n0:n0 + 128],
                                             pt3[:, :, n0:n0 + 128], m3[:])
                    if pend is not None:
                        pend()

                    def mk_ao(kt=kt, n0=n0, pt=pt,
                              st=(kt == 0), sp=(kt == nkt - 1)):
                        nc.tensor.matmul(
                            ao[:, n0:QT], v_sb[kt][:, he * C:(he + 1) * C],
                            pt[:, n0:QT], start=st, stop=sp)
                        nc.tensor.matmul(
                            ao[:, QT + n0:2 * QT],
                            v_sb[kt][:, ho * C:(ho + 1) * C],
                            pt[:, QT + n0:2 * QT], start=st, stop=sp)
                    pend = mk_ao
                pend()

                # normalize: 1/rowsum (row HD) via fast recip + gpsimd bcast
                am = patt.tile([128, QT], bf16, name=f"am{pg}{qi}", tag="am")
                att_m[(pg, qi)] = am
                srow = prr.tile([1, 2 * QT], f32, name=f"sr{pg}{qi}", tag="sr")
                nc.vector.tensor_copy(srow[:], ao[HD:HD + 1, :])
                nc.vector.reciprocal_approx_fast(srow[:], srow[:])
                rb = pr.tile([HD, 2 * QT], f32, name=f"rb{pg}{qi}", tag="r")
                nc.gpsimd.partition_broadcast(rb[:], srow[:], channels=HD)
                nc.vector.tensor_mul(am[0:64, :], ao[0:HD, 0:QT], rb[:, 0:QT])
                nc.vector.tensor_mul(am[64:128, :], ao[0:HD, QT:2 * QT],
                                     rb[:, QT:2 * QT])

            # ---- emission schedule ----
            # Prologue: v for kt 0-3 and q/k of pair 0, so attn(0,*) can
            # start; remaining projection work lands at block boundaries
            # where it overlaps the next block's exp pipeline.
            for u in (v_unit(0, 0), v_unit(0, 1), v_unit(1, 0), v_unit(1, 1)):
                u()
            for tg in range(4):
                qk_unit(0, 0, tg)()
                qk_unit(0, 1, tg)()

            boundary = {
                (0, 0): [v_unit(2, 0), v_unit(2, 1), v_unit(3, 0),
                         v_unit(3, 1), qk_unit(1, 0, 0), qk_unit(1, 0, 1)],
                (0, 1): [v_unit(4, 0), v_unit(4, 1), v_unit(5, 0),
                         v_unit(5, 1), qk_unit(1, 0, 2), qk_unit(1, 0, 3)],
                (0, 2): [v_unit(6, 0), v_unit(6, 1), v_unit(7, 0),
                         v_unit(7, 1), qk_unit(1, 1, 0), qk_unit(1, 1, 1)],
                (0, 3): [qk_unit(1, 1, 2), qk_unit(1, 1, 3)],
                (1, 0): [qk_unit(2, 0, 0), qk_unit(2, 0, 1)],
                (1, 1): [qk_unit(2, 0, 2), qk_unit(2, 0, 3)],
                (1, 2): [qk_unit(2, 1, 0), qk_unit(2, 1, 1)],
                (1, 3): [qk_unit(2, 1, 2), qk_unit(2, 1, 3)],
                (2, 0): [qk_unit(3, 0, 0), qk_unit(3, 0, 1)],
                (2, 1): [qk_unit(3, 0, 2), qk_unit(3, 0, 3)],
                (2, 2): [qk_unit(3, 1, 0), qk_unit(3, 1, 1)],
                (2, 3): [qk_unit(3, 1, 2), qk_unit(3, 1, 3)],
                (3, 0): [op_unit(0, dt) for dt in range(4)],
                (3, 1): [op_unit(0, dt) for dt in range(4, 8)]
                        + [op_unit(1, dt) for dt in range(4)],
                (3, 2): [op_unit(1, dt) for dt in range(4, 8)]
                        + [op_unit(2, dt) for dt in range(4)],
                (3, 3): [op_unit(2, dt) for dt in range(4, 8)],
            }
            for g in range(4):
                for qi in range(NQI):
                    attn_block(g, qi)
                    for u in boundary.get((g, qi), ()):
                        u()
            for dt in range(8):
                op_unit(3, dt)()

    nc.compile()
    return nc


def _get_nc():
    if "nc" not in _CACHE:
        _CACHE["nc"] = _build()
    return _CACHE["nc"]


def _prep_inputs(x, w_qkv, w_out, b_out):
    """Build the 8 per-core input maps (all payloads bf16)."""
    bf = ml_dtypes.bfloat16
    x = np.asarray(x, dtype=np.float32)
    w_qkv = np.asarray(w_qkv, dtype=np.float32)
    w_out = np.asarray(w_out, dtype=np.float32)

    tri = np.triu(np.ones((128, 128), dtype=np.float32))
    mask2 = np.tile(tri, (1, 2)).astype(bf)

    in_maps = []
    for c in range(8):
        b, hg = c // 2, c % 2
        cols = hg * F
        w_cat = np.concatenate([
            w_qkv[:, cols:cols + F],
            w_qkv[:, D + cols:D + cols + F],
            w_qkv[:, 2 * D + cols:2 * D + cols + F],
        ], axis=1)
        in_maps.append({
            "xT": np.ascontiguousarray(x[b].T).astype(bf),
            "w_sl": np.ascontiguousarray(w_cat).astype(bf),
            "wo_sl": np.ascontiguousarray(w_out[cols:cols + F, :]).astype(bf),
            "mask2": mask2,
        })
    return in_maps


def _run(inputs, trace=False):
    from concourse.bass_utils import run_bass_kernel_spmd

    nc = _get_nc()
    in_maps = _prep_inputs(**inputs)
    res = run_bass_kernel_spmd(nc, in_maps, core_ids=list(range(8)),
                               trace=trace)
    b_out = np.asarray(inputs["b_out"], dtype=np.float32)
    outs = []
    for b in range(B):
        o = (res.results[2 * b]["out"].astype(np.float32)
             + res.results[2 * b + 1]["out"].astype(np.float32))
        outs.append(o.T + b_out)
    full = np.stack(outs).astype(np.float32)
    return full, res


def kernel(x, w_qkv, w_out, b_out):
    full, _ = _run({"x": x, "w_qkv": w_qkv, "w_out": w_out, "b_out": b_out})
    return full


# revision 21
# speedup vs baseline: 1.2259x; 1.1468x over previous
"""Causal multi-head attention (B=4, S=2048, D=1024, H=16) on 8 NeuronCores.

Sharding: core c = (batch b = c//2, head-group hg = c%2). Each core computes
8 heads of one batch: QKV projection (bf16 matmuls), causal flash-style
attention (bf16 matmuls, exp-without-max softmax with a ones-column
denominator), and a row-parallel out-projection partial. Host sums the two
bf16 head-group partials per batch, adds bias, and transposes.

All HBM payloads are bf16. Layouts are feature-major ([feature, token])
except v (token-major) so attn@v needs no transposes. Head pairs are packed
into PE row groups (rows 0-63 / 64-127): the two K=64 score matmuls run
concurrently on PE row-tiles T0/T8. PSUM score tiles are 2 banks wide
(even head in columns 0-511, odd in 512-1023) so one ACT exp covers both
heads. Out-projection runs K=128 matmuls accumulating all 4 head pairs in
PSUM.

Scheduling: ACT exp throughput (~1µs per score tile) is the attention-phase
bottleneck, so projection/out-proj work is chopped into small filler units
(8 or 4 matmuls) and pumped between attention kt-tiles to fill the PE's
exp-wait gaps. DMA descriptor generation is spread across sync (weights)
and gpsimd (x, outputs) queues.
"""
import numpy as np

from contextlib import ExitStack

import ml_dtypes

B, S, D, H = 4, 2048, 1024, 16
HD = 64            # head dim
HPC = 8            # heads per core
F = HPC * HD       # 512 features per head-group
QT = 512           # q tile (free dim)
NQI = S // QT      # 4
NKT = S // 128     # 16
NDK = D // 128     # 8 contraction tiles for projections
SCALE = HD ** -0.5

_CACHE = {}


def _build():
    import concourse.bacc as bacc
    import concourse.tile as tile
    import concourse.mybir as mybir

    f32 = mybir.dt.float32
    bf16 = mybir.dt.bfloat16
    EXP = mybir.ActivationFunctionType.Exp

    nc = bacc.Bacc("TRN2", target_bir_lowering=False, debug=False)
    xT = nc.dram_tensor("xT", [D, S], bf16, kind="ExternalInput").ap()
    w_sl = nc.dram_tensor("w_sl", [D, 3 * F], bf16, kind="ExternalInput").ap()
    wo_sl = nc.dram_tensor("wo_sl", [F, D], bf16, kind="ExternalInput").ap()
    mask2 = nc.dram_tensor("mask2", [128, 256], bf16, kind="ExternalInput").ap()
    out = nc.dram_tensor("out", [D, S], bf16, kind="ExternalOutput").ap()

    with tile.TileContext(nc) as tc:
        with ExitStack() as ctx:
            misc = ctx.enter_context(tc.tile_pool(name="misc", bufs=1))
            mask_sb = misc.tile([128, 256], bf16, name="mask_sb", tag="mask")
            nc.sync.dma_start(mask_sb[:], mask2)

            pqk = ctx.enter_context(tc.tile_pool(name="pqk", bufs=1))
            pv = ctx.enter_context(tc.tile_pool(name="pv", bufs=1))
            patt = ctx.enter_context(tc.tile_pool(name="patt", bufs=16))
            pP = ctx.enter_context(tc.tile_pool(name="pP", bufs=4))
            pr = ctx.enter_context(tc.tile_pool(name="pr", bufs=2))
            prr = ctx.enter_context(tc.tile_pool(name="prr", bufs=1))
            pwo = ctx.enter_context(tc.tile_pool(name="pwo", bufs=1))
            pstg = ctx.enter_context(tc.tile_pool(name="pstg", bufs=3))
            xw = ctx.enter_context(tc.tile_pool(name="xw", bufs=1))

            q_sb = [pqk.tile([128, S], bf16, name=f"q{g}", tag=f"q{g}")
                    for g in range(4)]
            k_sb = [pqk.tile([128, S], bf16, name=f"k{g}", tag=f"k{g}")
                    for g in range(4)]
            v_sb = [pv.tile([128, HPC * (HD + 1)], bf16, name=f"v{t}",
                            tag=f"v{t}") for t in range(NKT)]

            psum = ctx.enter_context(
                tc.tile_pool(name="psum", bufs=2, space="PSUM"))

            # ---- input DMA, ordered by first use; x on the gpsimd queue,
            # weights on sync, so descriptor gen runs in parallel ----
            wv_t = []
            x_t = []
            wqk_t = []
            for kk in range(NDK):
                r0 = slice(kk * 128, (kk + 1) * 128)
                wv = xw.tile([128, F], bf16, name=f"wv{kk}", tag=f"wv{kk}")
                nc.sync.dma_start(wv[:], w_sl[r0, 2 * F:3 * F])
                wv_t.append(wv)
                x_t.append(xw.tile([128, S], bf16, name=f"x{kk}",
                                   tag=f"x{kk}"))
            for ch in range(4):
                cs = slice(ch * QT, (ch + 1) * QT)
                xq = nc.scalar if ch < 2 else nc.gpsimd
                for kk in range(NDK):
                    r0 = slice(kk * 128, (kk + 1) * 128)
                    xq.dma_start(x_t[kk][:, cs], xT[r0, cs])
                if ch == 1:
                    for kk in range(NDK):
                        r0 = slice(kk * 128, (kk + 1) * 128)
                        wq = xw.tile([128, 2 * F], bf16, name=f"wq{kk}",
                                     tag=f"wq{kk}")
                        nc.sync.dma_start(wq[:], w_sl[r0, 0:2 * F])
                        wqk_t.append(wq)
            wo_t = []
            for g in range(4):
                wt = pwo.tile([128, D], bf16, name=f"wo{g}", tag=f"wo{g}")
                nc.sync.dma_start(wt[:], wo_sl[g * 128:(g + 1) * 128, :])
                wo_t.append(wt)

            att_m = {}

            # ---- projection/out-proj filler units, pumped one at a time
            # into the PE gap between a tile's score matmuls and the
            # (lagged) attn@v matmuls. Budget-paced; at most one unit per
            # pump so PSUM slot rotation never waits on an in-flight unit.
            fillers = []
            state = {"budget": 0.0}

            def pump(ns):
                state["budget"] += ns
                if fillers and state["budget"] >= 1750:
                    state["budget"] -= 1750
                    fillers.pop(0)()

            def v_unit(t2, j):
                def emit():
                    tt = 2 * t2 + j
                    ps = psum.tile([128, QT], f32, name=f"pv{tt}", tag="sc")
                    for kk in range(NDK):
                        nc.tensor.matmul(
                            ps[:], x_t[kk][:, tt * 128:(tt + 1) * 128],
                            wv_t[kk][:],
                            start=(kk == 0), stop=(kk == NDK - 1))
                    vv = v_sb[tt].rearrange("p (h c) -> p h c", h=HPC)
                    pp = ps.rearrange("p (h c) -> p h c", h=HPC)
                    nc.vector.tensor_copy(vv[:, :, 0:HD], pp[:])
                    nc.vector.memset(vv[:, :, HD:HD + 1], 1.0)
                return emit

            def qk_unit(g, part, tg):
                def emit():
                    dest = q_sb if part == 0 else k_sb
                    fcol = part * F + g * 128
                    ps = psum.tile([128, QT], f32,
                                   name=f"pq{part}{g}{tg}", tag="sc")
                    for kk in range(NDK):
                        nc.tensor.matmul(
                            ps[:], wqk_t[kk][:, fcol:fcol + 128],
                            x_t[kk][:, tg * QT:(tg + 1) * QT],
                            start=(kk == 0), stop=(kk == NDK - 1))
                    nc.vector.tensor_copy(
                        dest[g][:, tg * QT:(tg + 1) * QT], ps[:])
                return emit

            def op_unit(qi, dt):
                def emit():
                    dcol = slice(dt * 128, dt * 128 + 128)
                    ps = psum.tile([128, QT], f32,
                                   name=f"op{dt}{qi}", tag="sc")
                    for pg in range(4):
                        nc.tensor.matmul(
                            ps[:], wo_t[pg][:, dcol], att_m[(pg, qi)][:],
                            start=(pg == 0), stop=(pg == 3))
                    s2 = pstg.tile([128, QT], bf16, name=f"s2{dt}{qi}",
                                   tag="s2")
                    nc.vector.tensor_copy(s2[:], ps[:])
                    nc.gpsimd.dma_start(
                        out[dt * 128:(dt + 1) * 128,
                            qi * QT:(qi + 1) * QT], s2[:])
                return emit

            def attn_block(pg, qi, rate=700):
                """Scores + exp + attn@v + normalize for head pair pg,
                q-range [qi*QT, (qi+1)*QT). attn@v for tile kt is emitted
                after the scores+exp of tile kt+1, so it never waits on
                the exp result (one-tile softmax lag). ~rate ns of filler
                is pumped into each tile's exp window."""
                nkt = 4 * qi + 4
                qs = qi * QT
                he, ho = 2 * pg, 2 * pg + 1
                C = HD + 1
                ao = psum.tile([HD + 1, 2 * QT], f32,
                               name=f"ao{pg}{qi}", tag="ao")
                pend = None
                for kt in range(nkt):
                    d = kt - 4 * qi
                    n0 = 0 if d < 0 else 128 * d
                    kcol = slice(kt * 128, kt * 128 + 128)
                    sc = psum.tile([128, 2 * QT], f32,
                                   name=f"sc{pg}{qi}{kt}", tag="sc")
                    nc.tensor.matmul(
                        sc[:, n0:QT], k_sb[pg][0:64, kcol],
                        q_sb[pg][0:64, qs + n0:qs + QT],
                        start=True, stop=True)
                    nc.tensor.matmul(
                        sc[:, QT + n0:2 * QT], k_sb[pg][64:128, kcol],
                        q_sb[pg][64:128, qs + n0:qs + QT],
                        start=True, stop=True)
                    pt = pP.tile([128, 2 * QT], bf16,
                                 name=f"pt{pg}{qi}{kt}", tag="P")
                    sc3 = sc.rearrange("p (h c) -> p h c", h=2)
                    pt3 = pt.rearrange("p (h c) -> p h c", h=2)
                    nc.scalar.activation(pt3[:, :, n0:QT], sc3[:, :, n0:QT],
                                         EXP, scale=SCALE)
                    if d >= 0:
                        m3 = mask_sb.rearrange("p (h c) -> p h c", h=2)
                        nc.vector.tensor_mul(pt3[:, :, n0:n0 + 128],
                                             pt3[:, :, n0:n0 + 128], m3[:])
                    pump(rate)
                    if pend is not None:
                        pend()

                    def mk_ao(kt=kt, n0=n0, pt=pt,
                              st=(kt == 0), sp=(kt == nkt - 1)):
                        nc.tensor.matmul(
                            ao[:, n0:QT], v_sb[kt][:, he * C:(he + 1) * C],
                            pt[:, n0:QT], start=st, stop=sp)
                        nc.tensor.matmul(
                            ao[:, QT + n0:2 * QT],
                            v_sb[kt][:, ho * C:(ho + 1) * C],
                            pt[:, QT + n0:2 * QT], start=st, stop=sp)
                    pend = mk_ao
                pend()

                # normalize: 1/rowsum (row HD) via fast recip + gpsimd bcast
                am = patt.tile([128, QT], bf16, name=f"am{pg}{qi}", tag="am")
                att_m[(pg, qi)] = am
                srow = prr.tile([1, 2 * QT], f32, name=f"sr{pg}{qi}", tag="sr")
                nc.vector.tensor_copy(srow[:], ao[HD:HD + 1, :])
                nc.vector.reciprocal_approx_fast(srow[:], srow[:])
                rb = pr.tile([HD, 2 * QT], f32, name=f"rb{pg}{qi}", tag="r")
                nc.gpsimd.partition_broadcast(rb[:], srow[:], channels=HD)
                nc.vector.tensor_mul(am[0:64, :], ao[0:HD, 0:QT], rb[:, 0:QT])
                nc.vector.tensor_mul(am[64:128, :], ao[0:HD, QT:2 * QT],
                                     rb[:, QT:2 * QT])

            # ---- emission schedule ----
            # Prologue: the minimum for attn(0,0): v for kt 0-3 and q/k
            # token-quarter 0 of pair 0. Everything else is filler,
            # ordered so tiles are ready ahead of the block needing them.
            for u in (v_unit(0, 0), v_unit(0, 1), v_unit(1, 0), v_unit(1, 1),
                      qk_unit(0, 0, 0), qk_unit(0, 1, 0)):
                u()

            fillers.extend([qk_unit(0, 0, 1), qk_unit(0, 1, 1),
                            v_unit(2, 0), v_unit(2, 1),
                            v_unit(3, 0), v_unit(3, 1),
                            qk_unit(0, 0, 2), qk_unit(0, 1, 2),
                            v_unit(4, 0), v_unit(4, 1),
                            v_unit(5, 0), v_unit(5, 1),
                            qk_unit(0, 0, 3), qk_unit(0, 1, 3),
                            v_unit(6, 0), v_unit(6, 1),
                            v_unit(7, 0), v_unit(7, 1)])
            for g in (1, 2, 3):
                fillers.extend(qk_unit(g, p, tg)
                               for tg in range(4) for p in (0, 1))

            for qi in range(NQI):
                attn_block(0, qi, rate=1000)
                pump(1500)
            for g in (1, 2):
                for qi in range(NQI):
                    attn_block(g, qi, rate=700)
                    pump(1500)
            for qi in range(NQI):
                attn_block(3, qi, rate=750)
                fillers.extend(op_unit(qi, dt) for dt in range(8))
                pump(1500)
            while fillers:
                fillers.pop(0)()

    nc.compile()
    return nc


def _get_nc():
    if "nc" not in _CACHE:
        _CACHE["nc"] = _build()
    return _CACHE["nc"]


def _prep_inputs(x, w_qkv, w_out, b_out):
    """Build the 8 per-core input maps (all payloads bf16)."""
    bf = ml_dtypes.bfloat16
    x = np.asarray(x, dtype=np.float32)
    w_qkv = np.asarray(w_qkv, dtype=np.float32)
    w_out = np.asarray(w_out, dtype=np.float32)

    tri = np.triu(np.ones((128, 128), dtype=np.float32))
    mask2 = np.tile(tri, (1, 2)).astype(bf)

    in_maps = []
    for c in range(8):
        b, hg = c // 2, c % 2
        cols = hg * F
        w_cat = np.concatenate([
            w_qkv[:, cols:cols + F],
            w_qkv[:, D + cols:D + cols + F],
            w_qkv[:, 2 * D + cols:2 * D + cols + F],
        ], axis=1)
        in_maps.append({
            "xT": np.ascontiguousarray(x[b].T).astype(bf),
            "w_sl": np.ascontiguousarray(w_cat).astype(bf),
            "wo_sl": np.ascontiguousarray(w_out[cols:cols + F, :]).astype(bf),
            "mask2": mask2,
        })
    return in_maps


def _run(inputs, trace=False):
    from concourse.bass_utils import run_bass_kernel_spmd

    nc = _get_nc()
    in_maps = _prep_inputs(**inputs)
    res = run_bass_kernel_spmd(nc, in_maps, core_ids=list(range(8)),
                               trace=trace)
    b_out = np.asarray(inputs["b_out"], dtype=np.float32)
    outs = []
    for b in range(B):
        o = (res.results[2 * b]["out"].astype(np.float32)
             + res.results[2 * b + 1]["out"].astype(np.float32))
        outs.append(o.T + b_out)
    full = np.stack(outs).astype(np.float32)
    return full, res


def kernel(x, w_qkv, w_out, b_out):
    full, _ = _run({"x": x, "w_qkv": w_qkv, "w_out": w_out, "b_out": b_out})
    return full
